# Initial kernel scaffold
#
"""Bi-LSTM Trainium2 kernel: B=64, T=256, D=512, H=512, fp32 I/O.

Sharding: 8 cores = 4 batch groups x 2 directions. Each core runs the full
time recurrence for its 16-sample shard in one direction (the backward
direction is handled by feeding that core a time-reversed input and
un-reversing its output on the host).

On-device layout is fully transposed: hidden/gate dims on SBUF partitions,
batch on the free dim. The recurrent matmul keeps the weight tile as the
stationary (lhsT) operand so the gate output lands transposed in PSUM,
which makes every elementwise op a [128, 64]-shaped op and removes any
per-step transposes.

Phase 1 precomputes gx[t] = x_t @ Wx + b for all t into DRAM scratch;
phase 2 runs the sequential recurrence g = gx[t] + h @ Wh plus the LSTM
cell elementwise. In the bf16 variant, gx is injected into PSUM with an
identity matmul (so the gate nonlinearities read PSUM directly, no DVE
adds), and phase-1 work is interleaved between recurrence steps so it
executes inside the PE gaps left by the elementwise tail.
"""

import sys

for _p in ("/opt/trn_rl_repo",):
    if _p not in sys.path:
        sys.path.append(_p)

import numpy as np
import ml_dtypes

import concourse.bass as bass
import concourse.mybir as mybir
from concourse import tile
from concourse.bass_utils import run_bass_kernel_spmd
from concourse.masks import make_identity

B, T, D, H = 64, 256, 512, 512
NCORES = 8
GROUPS = 4
BS = B // GROUPS          # batch rows per core
NK = H // 128             # contraction tiles over the hidden dim
NM = (4 * H) // 128       # output tiles over the gate dim
BLK_T = 32                # timesteps per phase-1 block
F32 = mybir.dt.float32
F32R = mybir.dt.float32r
BF16 = mybir.dt.bfloat16

# process gate blocks in [i, f, c, o] order so the output-gate chain is the
# only elementwise work left after the last matmul of a step
M_ORDER = list(range(0, 8)) + list(range(12, 16)) + list(range(8, 12))


def _patch_tail_drain():
    """This image's walrus rejects more than one sync-wait per engine
    instruction (and any wait on a self-loading 4-byte matmul). Tile
    attaches one wait per outstanding semaphore, so split the excess onto
    nofuse nops committed just before the instruction they guard (same
    engine -> identical semantics)."""
    import bass_rust
    from concourse.vector_clock import ScopedClock

    if getattr(tile.TileContext, "_drain_split_patched", False):
        return

    def _drain_and_barrier(self, tick_clock, wait_clock):
        drain_inst = self.nc.sync.drain()
        wait_clock.add_sem_waits(
            drain_inst.ins, ScopedClock({None: tick_clock.global_clock})
        )
        si = drain_inst.ins.sync_info
        if si is not None and len(si.on_wait) > 1:
            waits = list(si.on_wait)
            drain_inst.ins.sync_info = bass_rust.SyncInfo(
                on_wait=waits[:1], on_update=list(si.on_update)
            )
            for i in range(1, len(waits)):
                nop = self.nc.sync.nop(nofuse=True)
                nop.ins.sync_info = bass_rust.SyncInfo(
                    on_wait=waits[i : i + 1], on_update=[]
                )

        self.nc.all_engine_barrier()
        assert self.sems is not None
        popped = self.nc._tile_sem_poison_stack.pop()
        assert popped is self._sem_poison
        self.nc.clear_and_free_semaphores(list(self.sems.allocated().values()))
        self.nc.all_engine_barrier()

    tile.TileContext._drain_and_barrier = _drain_and_barrier

    orig_commit = tile.TileContext._commit_instruction

    def _commit_instruction(self, inst, lazy_reg_writes: bool = True):
        si = getattr(inst, "sync_info", None)
        limit = 0 if isinstance(inst, mybir.InstMatmult) else 1
        if (
            si is not None
            and len(si.on_wait) > limit
            and inst.engine != mybir.EngineType.Unassigned
        ):
            waits = list(si.on_wait)
            keep = waits[len(waits) - limit :] if limit else []
            for w in waits[: len(waits) - limit]:
                nop = mybir.InstNoOp(
                    name=f"I-{self.nc.next_id()}",
                    sync_info=mybir.SyncInfo(on_wait=[w], on_update=[]),
                    bass_nofuse=True,
                    engine=inst.engine,
                )
                orig_commit(self, nop, lazy_reg_writes=False)
            inst.sync_info = mybir.SyncInfo(
                on_wait=keep, on_update=list(si.on_update)
            )
        return orig_commit(self, inst, lazy_reg_writes)

    tile.TileContext._commit_instruction = _commit_instruction
    tile.TileContext._drain_split_patched = True


def build(recur_dt: str = "bf16", with_mask: bool = False, t_steps: int = T,
          gx_out: bool = False, p1_dt: str | None = None):
    """Emit the per-core SPMD module.

    recur_dt: dtype of Wh/h/gx for the recurrence ('bf16' or 'f32').
    p1_dt: matmul dtype of the x-projection ('f32r' or 'f32').
    """
    _patch_tail_drain()
    bf = recur_dt == "bf16"
    DT = BF16 if bf else F32
    GXDT = BF16 if bf else F32
    if p1_dt is None:
        p1_dt = "f32r" if bf else "f32"
    P1DT = F32R if p1_dt == "f32r" else F32
    blk_t = min(BLK_T, t_steps)
    nblk = t_steps // blk_t

    nc = bass.Bass("TRN2", target_bir_lowering=False, debug=False,
                   num_devices=NCORES)

    xT = nc.dram_tensor("xT", [D, t_steps * BS], P1DT, kind="ExternalInput")
    wx = nc.dram_tensor("wx", [D, 4 * H], P1DT, kind="ExternalInput")
    wh = nc.dram_tensor("wh", [H, 4 * H], DT, kind="ExternalInput")
    bt = nc.dram_tensor("bt", [128, NM], F32, kind="ExternalInput")
    msk = (
        nc.dram_tensor("msk", [t_steps, 128, NK * BS], F32,
                       kind="ExternalInput")
        if with_mask
        else None
    )
    hout = nc.dram_tensor("hout", [t_steps, 128, NK * BS], DT,
                          kind="ExternalOutput")
    ngx = 2 if bf else 1  # bf16 gx is stored as hi+lo bf16 pair
    gxd = nc.dram_tensor("gx_scratch", [128, t_steps, ngx * NM * BS], GXDT,
                         kind="ExternalOutput" if gx_out else "Internal")

    Act = mybir.ActivationFunctionType
    Alu = mybir.AluOpType

    with tile.TileContext(nc) as tc:
        with (
            tc.tile_pool(name="weights", bufs=1) as wpool,
            tc.tile_pool(name="state", bufs=1) as spool,
            tc.tile_pool(name="p1x", bufs=2) as xpool,
            tc.tile_pool(name="p1stg", bufs=2) as stgpool,
            tc.tile_pool(name="p1psum", bufs=2, space="PSUM") as p1ps,
            tc.tile_pool(name="p2psum", bufs=2, space="PSUM") as p2ps,
            tc.tile_pool(name="p2gx", bufs=4) as gxpool,
            tc.tile_pool(name="p2ew", bufs=2) as ewpool,
        ):
            wxs = wpool.tile([128, NK, 4 * H], P1DT)
            nc.gpsimd.dma_start(wxs[:], wx.ap().rearrange("(k p) n -> p k n", p=128))
            whs = wpool.tile([128, NK, 4 * H], DT)
            nc.gpsimd.dma_start(whs[:], wh.ap().rearrange("(k p) n -> p k n", p=128))
            bts = wpool.tile([128, NM], F32)
            nc.gpsimd.dma_start(bts[:], bt.ap())
            if bf:
                ident = wpool.tile([128, 128], BF16)
                make_identity(nc, ident[:])

            hT = spool.tile([128, NK, BS], DT)
            cT = spool.tile([128, NK, BS], F32)
            nc.vector.memset(hT[:], 0.0)
            nc.vector.memset(cT[:], 0.0)

            # ---- phase 1 machinery: gx[t] = x_t @ Wx + b ----
            xview = xT.ap().rearrange("(k p) n -> p k n", p=128)
            nfree = blk_t * BS
            p1_tiles: dict = {}
            anchor = [None]

            def p1_group(blk, m):
                """One m-tile of one phase-1 block: 4 matmuls + staging."""
                if m == 0:
                    xblk = xpool.tile([128, NK, nfree], P1DT, tag="xblk")
                    nc.gpsimd.dma_start(
                        xblk[:], xview[:, :, blk * nfree : (blk + 1) * nfree]
                    )
                    stg = stgpool.tile([128, blk_t, ngx, NM, BS], GXDT,
                                       tag="stg")
                    p1_tiles[blk] = (xblk, stg)
                xblk, stg = p1_tiles[blk]
                ps = p1ps.tile([128, nfree], F32, tag="p1ps")
                for k in range(NK):
                    mm = nc.tensor.matmul(
                        ps[:],
                        wxs[:, k, m * 128 : (m + 1) * 128],
                        xblk[:, k, :],
                        start=(k == 0),
                        stop=(k == NK - 1),
                    )
                    if k == 0 and anchor[0] is not None:
                        from concourse.bass import _add_dep_helper
                        _add_dep_helper(
                            mm.ins, anchor[0],
                            reason="pin p1 group behind its step",
                        )
                psv = ps[:].rearrange("p (t b) -> p t b", b=BS)
                if not bf:
                    if m % 2 == 0:
                        nc.vector.tensor_scalar(
                            stg[:, :, 0, m, :], psv, bts[:, m : m + 1], None,
                            Alu.add
                        )
                    else:
                        nc.scalar.activation(
                            stg[:, :, 0, m, :], psv, Act.Identity,
                            bias=bts[:, m : m + 1],
                        )
                else:
                    # hi = bf16(ps + b) on ACT; lo = bf16((ps + b) - hi) on DVE
                    nc.scalar.activation(
                        stg[:, :, 0, m, :], psv, Act.Identity,
                        bias=bts[:, m : m + 1],
                    )
                    nc.vector.scalar_tensor_tensor(
                        stg[:, :, 1, m, :], psv, bts[:, m : m + 1],
                        stg[:, :, 0, m, :], Alu.add, Alu.subtract,
                    )
                if m == NM - 1:
                    # split the store so early gx loads unblock sooner
                    qt = blk_t // 4 if blk_t % 4 == 0 else blk_t
                    for q in range(blk_t // qt):
                        nc.gpsimd.dma_start(
                            gxd.ap()[
                                :,
                                blk * blk_t + q * qt : blk * blk_t + (q + 1) * qt,
                                :,
                            ],
                            stg[:, q * qt : (q + 1) * qt].rearrange(
                                "p t g m b -> p t (g m b)"
                            ),
                        )
                    del p1_tiles[blk]

            # Interleave schedule: blocks 0/1 up front, block b's 16 groups
            # spread over steps [(b-2)*blk_t, (b-1)*blk_t) at 1 per 2 steps.
            sched: dict[int, list] = {}
            n_prologue = min(1, nblk)
            if bf:
                for b in range(n_prologue, nblk):
                    if b == 1:
                        slots = [2 * m for m in range(NM)]
                    elif b == 2:
                        slots = [16 + 3 * m for m in range(NM)]
                    else:
                        slots = [(b - 2) * blk_t + 3 * m for m in range(NM)]
                    for m, s in enumerate(slots):
                        sched.setdefault(s, []).append((b, m))
            else:
                n_prologue = nblk
            for b in range(n_prologue):
                for m in range(NM):
                    p1_group(b, m)

            # ---- phase 2: the recurrence ----
            for t in range(t_steps):
                gx = gxpool.tile([128, ngx, NM, BS], GXDT)
                nc.sync.dma_start(
                    gx[:],
                    gxd.ap()[:, t, :].rearrange("p (g m b) -> p g m b",
                                                b=BS, m=NM),
                )
                ps_if = p2ps.tile([128, 8, BS], F32, tag="ps_if")
                ps_c = p2ps.tile([128, 4, BS], F32, tag="ps_c")
                ps_o = p2ps.tile([128, 4, BS], F32, tag="ps_o")
                for m in M_ORDER:
                    if m < 8:
                        dst = ps_if[:, m, :]
                    elif m >= 12:
                        dst = ps_c[:, m - 12, :]
                    else:
                        dst = ps_o[:, m - 8, :]
                    if bf:
                        nc.tensor.matmul(dst, ident[:], gx[:, 0, m, :],
                                         start=True, stop=False)
                        nc.tensor.matmul(dst, ident[:], gx[:, 1, m, :],
                                         start=False, stop=False)
                    for k in range(NK):
                        mm = nc.tensor.matmul(
                            dst,
                            whs[:, k, m * 128 : (m + 1) * 128],
                            hT[:, k, :],
                            start=(not bf) and k == 0,
                            stop=(k == NK - 1),
                        )
                        anchor[0] = mm.ins
                    if m == 7:
                        sif = ewpool.tile([128, 8, BS], F32, tag="sif")
                        if bf:
                            nc.scalar.activation(sif[:], ps_if[:], Act.Sigmoid)
                        else:
                            nc.vector.tensor_tensor(
                                sif[:], ps_if[:], gx[:, 0, 0:8, :], Alu.add
                            )
                            nc.scalar.activation(sif[:], sif[:], Act.Sigmoid)
                    elif m == 15:
                        tcg = ewpool.tile([128, NK, BS], F32, tag="tcg")
                        if bf:
                            nc.scalar.activation(tcg[:], ps_c[:], Act.Tanh)
                        else:
                            nc.vector.tensor_tensor(
                                tcg[:], ps_c[:], gx[:, 0, 12:16, :], Alu.add
                            )
                            nc.scalar.activation(tcg[:], tcg[:], Act.Tanh)
                        t1 = ewpool.tile([128, NK, BS], F32, tag="t1")
                        nc.vector.tensor_tensor(
                            t1[:], sif[:, 4:8, :], cT[:], Alu.mult
                        )
                        t2 = ewpool.tile([128, NK, BS], F32, tag="t2")
                        nc.vector.tensor_tensor(
                            t2[:], sif[:, 0:4, :], tcg[:], Alu.mult
                        )
                        if with_mask:
                            cn = ewpool.tile([128, NK, BS], F32, tag="cn")
                            nc.vector.tensor_tensor(cn[:], t1[:], t2[:], Alu.add)
                            mt = ewpool.tile([128, NK * BS], F32, tag="mt")
                            nc.gpsimd.dma_start(mt[:], msk[t])
                            mtv = mt[:].rearrange("p (k b) -> p k b", b=BS)
                            cd = ewpool.tile([128, NK, BS], F32, tag="cd")
                            nc.vector.tensor_tensor(cd[:], cn[:], cT[:],
                                                    Alu.subtract)
                            nc.vector.tensor_tensor(cd[:], cd[:], mtv, Alu.mult)
                            nc.vector.tensor_tensor(cT[:], cT[:], cd[:], Alu.add)
                        else:
                            nc.vector.tensor_tensor(cT[:], t1[:], t2[:], Alu.add)
                        thc = ewpool.tile([128, NK, BS], F32, tag="thc")
                        nc.scalar.activation(thc[:], cT[:], Act.Tanh)
                # output gate chain + h update
                og = ewpool.tile([128, NK, BS], F32, tag="og")
                if bf:
                    nc.scalar.activation(og[:], ps_o[:], Act.Sigmoid)
                else:
                    nc.vector.tensor_tensor(og[:], ps_o[:], gx[:, 0, 8:12, :],
                                            Alu.add)
                    nc.scalar.activation(og[:], og[:], Act.Sigmoid)
                # threshold(o, 0.4): o if o > 0.4 else 0
                nc.vector.scalar_tensor_tensor(
                    og[:], og[:], 0.4, og[:], Alu.is_gt, Alu.mult
                )
                if with_mask:
                    hn = ewpool.tile([128, NK, BS], F32, tag="hn")
                    nc.vector.tensor_tensor(hn[:], og[:], thc[:], Alu.mult)
                    hd = ewpool.tile([128, NK, BS], F32, tag="hd")
                    nc.vector.tensor_tensor(hd[:], hn[:], hT[:], Alu.subtract)
                    nc.vector.tensor_tensor(hd[:], hd[:], mtv, Alu.mult)
                    nc.vector.tensor_tensor(hT[:], hT[:], hd[:], Alu.add)
                else:
                    nc.vector.tensor_tensor(hT[:], og[:], thc[:], Alu.mult)
                nc.sync.dma_start(hout[t], hT[:].rearrange("p k b -> p (k b)"))

                for b, m in sched.get(t, ()):
                    p1_group(b, m)
    return nc


_BUILD_CACHE: dict = {}


def _get_module(recur_dt: str, with_mask: bool, t_steps: int = T,
                p1_dt: str | None = None):
    key = (recur_dt, with_mask, t_steps, p1_dt)
    if key not in _BUILD_CACHE:
        _BUILD_CACHE[key] = build(recur_dt, with_mask, t_steps, p1_dt=p1_dt)
    return _BUILD_CACHE[key]


def _make_in_maps(x, mask, Wf, bf, Wb, bb, recur_dt: str, with_mask: bool,
                  t_steps: int = T):
    DTnp = ml_dtypes.bfloat16 if recur_dt == "bf16" else np.float32
    ws = {}
    for d, (W, bias) in enumerate(((Wf, bf), (Wb, bb))):
        W = np.asarray(W, np.float32)
        ws[d] = (
            np.ascontiguousarray(W[H:]),                 # wx (x rows), f32
            np.ascontiguousarray(W[:H].astype(DTnp)),    # wh (h rows)
            np.ascontiguousarray(
                np.asarray(bias, np.float32).reshape(NM, 128).T
            ),
        )
    in_maps = []
    for core in range(NCORES):
        g, d = core // 2, core % 2
        xs = np.asarray(x[g * BS : (g + 1) * BS, :t_steps], np.float32)
        ms = np.asarray(mask[g * BS : (g + 1) * BS, :t_steps], np.float32)
        if d == 1:
            xs = xs[:, ::-1]
            ms = ms[:, ::-1]
        # xT[dd, t*BS + b] = xs[b, t, dd]
        xTv = np.ascontiguousarray(
            xs.transpose(2, 1, 0).reshape(D, t_steps * BS)
        )
        wxv, whv, btv = ws[d]
        m = {"xT": xTv, "wx": wxv, "wh": whv, "bt": btv}
        if with_mask:
            m["msk"] = np.ascontiguousarray(
                np.broadcast_to(
                    ms.T[:, None, None, :], (t_steps, 128, NK, BS)
                ).reshape(t_steps, 128, NK * BS)
            )
        in_maps.append(m)
    return in_maps


def _assemble(results, t_steps: int = T):
    out = np.empty((B, t_steps, 2 * H), np.float32)
    for core in range(NCORES):
        g, d = core // 2, core % 2
        h = np.asarray(results[core]["hout"], np.float32)  # [t, 128, NK*BS]
        h = h.reshape(t_steps, 128, NK, BS).transpose(3, 0, 2, 1)  # [b,t,k,p]
        h = h.reshape(BS, t_steps, H)
        if d == 1:
            h = h[:, ::-1]
        out[g * BS : (g + 1) * BS, :, d * H : (d + 1) * H] = h
    return out


def run(x, mask, Wf, bf, Wb, bb, recur_dt="bf16", trace=False, t_steps: int = T,
        p1_dt: str | None = None, **spmd_kwargs):
    with_mask = not bool(np.all(np.asarray(mask) == 1.0))
    nc = _get_module(recur_dt, with_mask, t_steps, p1_dt)
    in_maps = _make_in_maps(x, mask, Wf, bf, Wb, bb, recur_dt, with_mask,
                            t_steps)
    res = run_bass_kernel_spmd(
        nc, in_maps, list(range(NCORES)), trace=trace, **spmd_kwargs
    )
    return _assemble(res.results, t_steps), res


def kernel(x, mask, Wf, bf, Wb, bb):
    out, _ = run(x, mask, Wf, bf, Wb, bb)
    return out



# revision 38
# speedup vs baseline: 1.0277x; 1.0277x over previous
"""Bi-LSTM Trainium2 kernel: B=64, T=256, D=512, H=512, fp32 I/O.

Sharding: 8 cores = 4 batch groups x 2 directions. Each core runs the full
time recurrence for its 16-sample shard in one direction (the backward
direction is handled by feeding that core a time-reversed input and
un-reversing its output on the host).

On-device layout is fully transposed: hidden/gate dims on SBUF partitions,
batch on the free dim. The recurrent matmul keeps the weight tile as the
stationary (lhsT) operand so the gate output lands transposed in PSUM,
which makes every elementwise op a [128, 64]-shaped op and removes any
per-step transposes.

Phase 1 precomputes gx[t] = x_t @ Wx + b for all t into DRAM scratch;
phase 2 runs the sequential recurrence g = gx[t] + h @ Wh plus the LSTM
cell elementwise. gx is injected into PSUM with identity matmuls (bf16
hi+lo pair) so the gate nonlinearities read PSUM directly.

Step scheduling: all gx-injection matmuls for step t (h-independent) are
emitted BEFORE the h-dependent Wh matmuls, and phase-1 groups are emitted
between them, so the in-order PE queue drains gx/phase-1 work during the
previous step's serial elementwise tail instead of idling. gx is loaded
in 8-step batches, h lives in a 32-slot SBUF ring with chunked output
DMA (breaking the per-step write-after-read stall on the h store), and
the three gate PSUM accumulators each own a full PSUM bank.
"""

import sys

for _p in ("/opt/trn_rl_repo",):
    if _p not in sys.path:
        sys.path.append(_p)

import numpy as np
import ml_dtypes

import concourse.bass as bass
import concourse.mybir as mybir
from concourse import tile
from concourse.bass_utils import run_bass_kernel_spmd
from concourse.masks import make_identity

B, T, D, H = 64, 256, 512, 512
NCORES = 8
GROUPS = 4
BS = B // GROUPS          # batch rows per core
NK = H // 128             # contraction tiles over the hidden dim
NM = (4 * H) // 128       # output tiles over the gate dim
BLK_T = 32                # timesteps per phase-1 block
GXB = 8                   # timesteps per gx load batch
RING = 32                 # h ring slots
CHUNK = 16                # timesteps per hout store chunk
F32 = mybir.dt.float32
F32R = mybir.dt.float32r
BF16 = mybir.dt.bfloat16

# wh matmul order over gate blocks: [i] (m 0..3), [c] (m 12..15),
# [f] (m 4..7), [o] (m 8..11). sigmoid(i) and tanh(cg) -- the two inputs
# of the critical t2 product -- complete mid-dense; sigmoid(f) feeds only
# the off-chain t1; o stays last so just the output-gate chain trails.
M_ORDER = (list(range(0, 4)) + list(range(12, 16)) + list(range(4, 8))
           + list(range(8, 12)))


def _patch_tail_drain():
    """This image's walrus rejects more than one sync-wait per engine
    instruction (and any wait on a self-loading 4-byte matmul). Tile
    attaches one wait per outstanding semaphore, so split the excess onto
    nofuse nops committed just before the instruction they guard (same
    engine -> identical semantics)."""
    import bass_rust
    from concourse.vector_clock import ScopedClock

    if getattr(tile.TileContext, "_drain_split_patched", False):
        return

    def _drain_and_barrier(self, tick_clock, wait_clock):
        drain_inst = self.nc.sync.drain()
        wait_clock.add_sem_waits(
            drain_inst.ins, ScopedClock({None: tick_clock.global_clock})
        )
        si = drain_inst.ins.sync_info
        if si is not None and len(si.on_wait) > 1:
            waits = list(si.on_wait)
            drain_inst.ins.sync_info = bass_rust.SyncInfo(
                on_wait=waits[:1], on_update=list(si.on_update)
            )
            for i in range(1, len(waits)):
                nop = self.nc.sync.nop(nofuse=True)
                nop.ins.sync_info = bass_rust.SyncInfo(
                    on_wait=waits[i : i + 1], on_update=[]
                )

        self.nc.all_engine_barrier()
        assert self.sems is not None
        popped = self.nc._tile_sem_poison_stack.pop()
        assert popped is self._sem_poison
        self.nc.clear_and_free_semaphores(list(self.sems.allocated().values()))
        self.nc.all_engine_barrier()

    tile.TileContext._drain_and_barrier = _drain_and_barrier

    orig_commit = tile.TileContext._commit_instruction

    def _commit_instruction(self, inst, lazy_reg_writes: bool = True):
        si = getattr(inst, "sync_info", None)
        if isinstance(inst, mybir.InstMatmult):
            # only self-loading 4-byte matmuls reject a fused wait
            wdt = getattr(inst.ins[1], "dtype", None)
            limit = 0 if wdt in (mybir.dt.float32, mybir.dt.float32r) else 1
        else:
            limit = 1
        if (
            si is not None
            and len(si.on_wait) > limit
            and inst.engine != mybir.EngineType.Unassigned
        ):
            waits = list(si.on_wait)
            keep = waits[len(waits) - limit :] if limit else []
            for w in waits[: len(waits) - limit]:
                nop = mybir.InstNoOp(
                    name=f"I-{self.nc.next_id()}",
                    sync_info=mybir.SyncInfo(on_wait=[w], on_update=[]),
                    bass_nofuse=True,
                    engine=inst.engine,
                )
                orig_commit(self, nop, lazy_reg_writes=False)
            inst.sync_info = mybir.SyncInfo(
                on_wait=keep, on_update=list(si.on_update)
            )
        return orig_commit(self, inst, lazy_reg_writes)

    tile.TileContext._commit_instruction = _commit_instruction
    tile.TileContext._drain_split_patched = True


def build(recur_dt: str = "bf16", with_mask: bool = False, t_steps: int = T,
          gx_out: bool = False, p1_dt: str | None = None):
    """Emit the per-core SPMD module."""
    _patch_tail_drain()
    if p1_dt is None:
        p1_dt = "f32r"
    P1DT = F32R if p1_dt == "f32r" else F32
    blk_t = min(BLK_T, t_steps)
    nblk = t_steps // blk_t
    gxb = min(GXB, t_steps)
    nbatch = t_steps // gxb
    ring_n = min(RING, t_steps)
    chunk = min(CHUNK, t_steps)

    nc = bass.Bass("TRN2", target_bir_lowering=False, debug=False,
                   num_devices=NCORES)

    xT = nc.dram_tensor("xT", [D, t_steps * BS], P1DT, kind="ExternalInput")
    wx = nc.dram_tensor("wx", [D, 4 * H], P1DT, kind="ExternalInput")
    wh = nc.dram_tensor("wh", [H, 4 * H], BF16, kind="ExternalInput")
    bt = nc.dram_tensor("bt", [128, NM], F32, kind="ExternalInput")
    msk = (
        nc.dram_tensor("msk", [t_steps, 128, NK * BS], F32,
                       kind="ExternalInput")
        if with_mask
        else None
    )
    hout = nc.dram_tensor("hout", [128, t_steps, NK * BS], BF16,
                          kind="ExternalOutput")
    ngx = 2  # gx stored as bf16 hi+lo pair
    gxd = nc.dram_tensor("gx_scratch", [128, t_steps, ngx * NM * BS], BF16,
                         kind="ExternalOutput" if gx_out else "Internal")

    Act = mybir.ActivationFunctionType
    Alu = mybir.AluOpType

    with tile.TileContext(nc) as tc:
        with (
            tc.tile_pool(name="weights", bufs=1) as wpool,
            tc.tile_pool(name="state", bufs=1) as spool,
            tc.tile_pool(name="p1x", bufs=2) as xpool,
            tc.tile_pool(name="p1stg", bufs=2) as stgpool,
            tc.tile_pool(name="p1psum", bufs=2, space="PSUM") as p1ps,
            tc.tile_pool(name="p2psum", bufs=2, space="PSUM") as p2ps,
            tc.tile_pool(name="p2psum1", bufs=1, space="PSUM") as p2ps1,
            tc.tile_pool(name="p2gx", bufs=2) as gxpool,
            tc.tile_pool(name="p2ew", bufs=2) as ewpool,
        ):
            wxs = wpool.tile([128, NK, 4 * H], P1DT)
            nc.gpsimd.dma_start(wxs[:], wx.ap().rearrange("(k p) n -> p k n", p=128))
            whs = wpool.tile([128, NK, 4 * H], BF16)
            nc.gpsimd.dma_start(whs[:], wh.ap().rearrange("(k p) n -> p k n", p=128))
            bts = wpool.tile([128, NM], F32)
            nc.gpsimd.dma_start(bts[:], bt.ap())
            ident = wpool.tile([128, 128], BF16)
            make_identity(nc, ident[:])

            # h ring: slot t%ring_n holds h after step t; slot ring_n-1 is
            # the zero initial state.
            hring = spool.tile([128, ring_n, NK, BS], BF16)
            cT = spool.tile([128, NK, BS], F32)
            nc.vector.memset(hring[:, ring_n - 1], 0.0)
            nc.vector.memset(cT[:], 0.0)

            # ---- phase 1 machinery: gx[t] = x_t @ Wx + b ----
            xview = xT.ap().rearrange("(k p) n -> p k n", p=128)
            nfree = blk_t * BS
            p1_tiles: dict = {}

            def xblk_load(b):
                xblk = xpool.tile([128, NK, nfree], P1DT, tag="xblk")
                nc.gpsimd.dma_start(
                    xblk[:], xview[:, :, b * nfree : (b + 1) * nfree]
                )
                return xblk

            xblk_tiles: dict = {}

            p1_ps: dict = {}

            def p1_mm_half(blk, m, half):
                """Half of one p1 m-tile: 2 matmuls (~0.6us of PE), sized
                to fit a per-step tail window."""
                if m == 0 and half == 0:
                    if blk not in xblk_tiles:
                        xblk_tiles[blk] = xblk_load(blk)
                    stg = stgpool.tile([128, blk_t, ngx, NM, BS], BF16,
                                       tag="stg")
                    p1_tiles[blk] = (xblk_tiles[blk], stg)
                xblk, stg = p1_tiles[blk]
                if half == 0:
                    ps = p1ps.tile([128, nfree], F32, tag="p1ps")
                    p1_ps[(blk, m)] = ps
                else:
                    ps = p1_ps[(blk, m)]
                for k in ((0, 1) if half == 0 else (2, 3)):
                    nc.tensor.matmul(
                        ps[:],
                        wxs[:, k, m * 128 : (m + 1) * 128],
                        xblk[:, k, :],
                        start=(k == 0),
                        stop=(k == NK - 1),
                    )

            def p1_stage(blk, m):
                """Bias + hi/lo staging of one finished p1 m-tile. Emitted
                post-chain so the ACT/DVE work runs during the next dense
                phase instead of in front of the chain ops."""
                xblk, stg = p1_tiles[blk]
                ps = p1_ps.pop((blk, m))
                psv = ps[:].rearrange("p (t b) -> p t b", b=BS)
                # hi = bf16(ps + b) on ACT; lo = bf16((ps + b) - hi) on DVE
                nc.scalar.activation(
                    stg[:, :, 0, m, :], psv, Act.Identity,
                    bias=bts[:, m : m + 1],
                )
                nc.vector.scalar_tensor_tensor(
                    stg[:, :, 1, m, :], psv, bts[:, m : m + 1],
                    stg[:, :, 0, m, :], Alu.add, Alu.subtract,
                )
                if m == NM - 1:
                    # store in gxb-sized pieces so batched gx loads unblock
                    qt = gxb if blk_t % gxb == 0 else blk_t
                    for q in range(blk_t // qt):
                        nc.gpsimd.dma_start(
                            gxd.ap()[
                                :,
                                blk * blk_t + q * qt : blk * blk_t + (q + 1) * qt,
                                :,
                            ],
                            stg[:, q * qt : (q + 1) * qt].rearrange(
                                "p t g m b -> p t (g m b)"
                            ),
                        )
                    del p1_tiles[blk]
                    del xblk_tiles[blk]
                    # prefetch the x block two ahead into the freed buffer
                    if blk + 2 < nblk and blk + 2 not in xblk_tiles:
                        xblk_tiles[blk + 2] = xblk_load(blk + 2)

            # p1 schedule: block 0 in the prologue; block 1 as full groups
            # in the first 16 steps (staged same-step); block b>=2 as
            # half-groups, one per step, with staging emitted post-chain of
            # the half1 step. The staging read then completes a full step
            # before its PSUM buffer is reallocated, so it never blocks the
            # PE queue.
            sched: dict[int, list] = {}
            sched_stage: dict[int, list] = {}
            for b in range(1, nblk):
                if b == 1:
                    for m in range(NM):
                        sched.setdefault(m, []).append((b, m, 0))
                        sched.setdefault(m, []).append((b, m, 1))
                        sched_stage.setdefault(m, []).append((b, m))
                else:
                    base = (b - 2) * blk_t + 16
                    for m in range(NM):
                        sched.setdefault(base + 2 * m, []).append((b, m, 0))
                        sched.setdefault(base + 2 * m + 1, []).append(
                            (b, m, 1)
                        )
                        sched_stage.setdefault(base + 2 * m + 1, []).append(
                            (b, m)
                        )
            xblk_tiles[0] = xblk_load(0)
            if nblk > 1:
                xblk_tiles[1] = xblk_load(1)
            for m in range(NM):
                p1_mm_half(0, m, 0)
                p1_mm_half(0, m, 1)
                p1_stage(0, m)

            # ---- phase 2: the recurrence ----
            # gx batch loads: batch i covers steps [i*gxb, (i+1)*gxb).
            gx_tiles: dict = {}

            def gx_load(i):
                g = gxpool.tile([128, gxb, ngx, NM, BS], BF16, tag="gx")
                nc.sync.dma_start(
                    g[:],
                    gxd.ap()[:, i * gxb : (i + 1) * gxb, :].rearrange(
                        "p t (g m b) -> p t g m b", b=BS, m=NM
                    ),
                )
                gx_tiles[i] = g

            gx_load(0)

            hchunks: list = []

            for t in range(t_steps):
                if t % gxb == 0 and (t // gxb) + 1 < nbatch:
                    gx_load((t // gxb) + 1)
                gx = gx_tiles[t // gxb][:, t % gxb]

                # gate PSUM accumulators; each gets a full 2KB bank so
                # ACT reads never share a bank with in-flight matmuls.
                # i and f get separate single-buffered banks so sigmoid(i)
                # can fire as soon as the i matmuls finish (their next-step
                # ident WAR resolves mid-dense, so one buffer suffices).
                ps_i = p2ps1.tile([128, 512], F32, tag="ps_i")
                ps_f = p2ps1.tile([128, 512], F32, tag="ps_f")
                ps_c = p2ps.tile([128, 512], F32, tag="ps_c")
                ps_o = p2ps.tile([128, 512], F32, tag="ps_o")
                ps_iv = ps_i[:, 0:64].rearrange("p (m b) -> p m b", b=BS)
                ps_fv = ps_f[:, 0:64].rearrange("p (m b) -> p m b", b=BS)
                ps_cv = ps_c[:, 0:64].rearrange("p (m b) -> p m b", b=BS)
                ps_ov = ps_o[:, 0:64].rearrange("p (m b) -> p m b", b=BS)

                def dst_of(m):
                    if m < 4:
                        return ps_iv[:, m, :]
                    if m < 8:
                        return ps_fv[:, m - 4, :]
                    if m >= 12:
                        return ps_cv[:, m - 12, :]
                    return ps_ov[:, m - 8, :]

                # h-independent gx injections first: they run during the
                # previous step's elementwise tail. The identity stationary
                # is shared and each PSUM bank's gate block is contiguous in
                # gx, so one wide matmul per (bank, hi/lo) injects the whole
                # block: 6 matmuls instead of 32.
                for dst, m0, m1 in (
                    (ps_i[:, 0:64], 0, 4),
                    (ps_f[:, 0:64], 4, 8),
                    (ps_o[:, 0:64], 8, 12),
                    (ps_c[:, 0:64], 12, 16),
                ):
                    for g in range(ngx):
                        nc.tensor.matmul(dst, ident[:], gx[:, g, m0:m1, :],
                                         start=(g == 0), stop=False,
                                         skip_group_check=True)

                # phase-1 matmuls scheduled at this step also fill the tail
                for b, m, half in sched.get(t, ()):
                    p1_mm_half(b, m, half)

                # h-dependent recurrence matmuls
                hprev = hring[:, (t + ring_n - 1) % ring_n]
                for m in M_ORDER:
                    dst = dst_of(m)
                    for k in range(NK):
                        nc.tensor.matmul(
                            dst,
                            whs[:, k, m * 128 : (m + 1) * 128],
                            hprev[:, k, :],
                            start=False,
                            stop=(k == NK - 1) and m in (3, 7, 11, 15),
                            skip_group_check=True,
                        )
                    if m == 3:
                        # sigmoid(i) as soon as the i-gate bank is complete
                        sgi = ewpool.tile([128, NK, BS], F32, tag="sgi")
                        nc.scalar.activation(sgi[:], ps_iv[:], Act.Sigmoid)
                    elif m == 15:
                        tcg = ewpool.tile([128, NK, BS], F32, tag="tcg")
                        nc.scalar.activation(tcg[:], ps_cv[:], Act.Tanh)
                        t2 = ewpool.tile([128, NK, BS], F32, tag="t2")
                        nc.vector.tensor_tensor(
                            t2[:], sgi[:], tcg[:], Alu.mult
                        )
                    elif m == 7:
                        sgf = ewpool.tile([128, NK, BS], F32, tag="sgf")
                        nc.scalar.activation(sgf[:], ps_fv[:], Act.Sigmoid)
                        t1 = ewpool.tile([128, NK, BS], F32, tag="t1")
                        nc.vector.tensor_tensor(
                            t1[:], sgf[:], cT[:], Alu.mult
                        )
                        if with_mask:
                            cn = ewpool.tile([128, NK, BS], F32, tag="cn")
                            nc.vector.tensor_tensor(cn[:], t1[:], t2[:], Alu.add)
                            mt = ewpool.tile([128, NK * BS], F32, tag="mt")
                            nc.gpsimd.dma_start(mt[:], msk[t])
                            mtv = mt[:].rearrange("p (k b) -> p k b", b=BS)
                            cd = ewpool.tile([128, NK, BS], F32, tag="cd")
                            nc.vector.tensor_tensor(cd[:], cn[:], cT[:],
                                                    Alu.subtract)
                            nc.vector.tensor_tensor(cd[:], cd[:], mtv, Alu.mult)
                            nc.vector.tensor_tensor(cT[:], cT[:], cd[:], Alu.add)
                        else:
                            nc.vector.tensor_tensor(cT[:], t1[:], t2[:], Alu.add)
                # output gate chain + h update; thc is emitted after sig_o
                # so the ACT queue runs [sgi, tcg, sgf, sig_o, thc] in
                # dependency order.
                og = ewpool.tile([128, NK, BS], F32, tag="og")
                nc.scalar.activation(og[:], ps_ov[:], Act.Sigmoid)
                # threshold(o, 0.4): o if o > 0.4 else 0
                nc.vector.scalar_tensor_tensor(
                    og[:], og[:], 0.4, og[:], Alu.is_gt, Alu.mult
                )
                thc = ewpool.tile([128, NK, BS], F32, tag="thc")
                nc.scalar.activation(thc[:], cT[:], Act.Tanh)
                hcur = hring[:, t % ring_n]
                if with_mask:
                    hn = ewpool.tile([128, NK, BS], F32, tag="hn")
                    nc.vector.tensor_tensor(hn[:], og[:], thc[:], Alu.mult)
                    hd = ewpool.tile([128, NK, BS], F32, tag="hd")
                    nc.vector.tensor_tensor(hd[:], hn[:], hprev[:], Alu.subtract)
                    nc.vector.tensor_tensor(hd[:], hd[:], mtv, Alu.mult)
                    nc.vector.tensor_tensor(hcur[:], hprev[:], hd[:], Alu.add)
                else:
                    nc.vector.tensor_tensor(hcur[:], og[:], thc[:], Alu.mult)

                # deferred p1 staging: runs during the next dense phase
                for b, m in sched_stage.get(t, ()):
                    p1_stage(b, m)

                if (t + 1) % chunk == 0:
                    c0 = t + 1 - chunk
                    nc.gpsimd.dma_start(
                        hout.ap()[:, c0 : t + 1, :],
                        hring[:, c0 % ring_n : c0 % ring_n + chunk].rearrange(
                            "p t k b -> p t (k b)"
                        ),
                    )
    return nc


_BUILD_CACHE: dict = {}


def _get_module(recur_dt: str, with_mask: bool, t_steps: int = T,
                p1_dt: str | None = None):
    key = (recur_dt, with_mask, t_steps, p1_dt)
    if key not in _BUILD_CACHE:
        _BUILD_CACHE[key] = build(recur_dt, with_mask, t_steps, p1_dt=p1_dt)
    return _BUILD_CACHE[key]


def _make_in_maps(x, mask, Wf, bf, Wb, bb, recur_dt: str, with_mask: bool,
                  t_steps: int = T):
    ws = {}
    for d, (W, bias) in enumerate(((Wf, bf), (Wb, bb))):
        W = np.asarray(W, np.float32)
        ws[d] = (
            np.ascontiguousarray(W[H:]),                 # wx (x rows), f32
            np.ascontiguousarray(W[:H].astype(ml_dtypes.bfloat16)),  # wh
            np.ascontiguousarray(
                np.asarray(bias, np.float32).reshape(NM, 128).T
            ),
        )
    in_maps = []
    for core in range(NCORES):
        g, d = core // 2, core % 2
        xs = np.asarray(x[g * BS : (g + 1) * BS, :t_steps], np.float32)
        ms = np.asarray(mask[g * BS : (g + 1) * BS, :t_steps], np.float32)
        if d == 1:
            xs = xs[:, ::-1]
            ms = ms[:, ::-1]
        # xT[dd, t*BS + b] = xs[b, t, dd]
        xTv = np.ascontiguousarray(
            xs.transpose(2, 1, 0).reshape(D, t_steps * BS)
        )
        wxv, whv, btv = ws[d]
        m = {"xT": xTv, "wx": wxv, "wh": whv, "bt": btv}
        if with_mask:
            m["msk"] = np.ascontiguousarray(
                np.broadcast_to(
                    ms.T[:, None, None, :], (t_steps, 128, NK, BS)
                ).reshape(t_steps, 128, NK * BS)
            )
        in_maps.append(m)
    return in_maps


def _assemble(results, t_steps: int = T):
    out = np.empty((B, t_steps, 2 * H), np.float32)
    for core in range(NCORES):
        g, d = core // 2, core % 2
        h = np.asarray(results[core]["hout"], np.float32)  # [128, t, NK*BS]
        h = h.reshape(128, t_steps, NK, BS).transpose(3, 1, 2, 0)  # [b,t,k,p]
        h = h.reshape(BS, t_steps, H)
        if d == 1:
            h = h[:, ::-1]
        out[g * BS : (g + 1) * BS, :, d * H : (d + 1) * H] = h
    return out


def run(x, mask, Wf, bf, Wb, bb, recur_dt="bf16", trace=False, t_steps: int = T,
        p1_dt: str | None = None, **spmd_kwargs):
    with_mask = not bool(np.all(np.asarray(mask) == 1.0))
    nc = _get_module(recur_dt, with_mask, t_steps, p1_dt)
    in_maps = _make_in_maps(x, mask, Wf, bf, Wb, bb, recur_dt, with_mask,
                            t_steps)
    res = run_bass_kernel_spmd(
        nc, in_maps, list(range(NCORES)), trace=trace, **spmd_kwargs
    )
    return _assemble(res.results, t_steps), res


def kernel(x, mask, Wf, bf, Wb, bb):
    out, _ = run(x, mask, Wf, bf, Wb, bb)
    return out
